# revision 1
# baseline (speedup 1.0000x reference)
"""DCNv3 deformable conv kernel for Trainium2, 8-core data-parallel.

Contract: kernel(**inputs) takes the FULL inputs (as produced by
setup_inputs) and returns the FULL output [B, 64, H, W] float32.

Strategy (per core = half of one batch image = 64 output rows):
- NCHW input slices put channels on partitions; all projections and the
  3x3 conv (taps paired 2-per-matmul on K=128) run as fp16 PE matmuls.
- Bilinear sampling with |offset| < 1 is EXACTLY a 5x5 window stencil:
    out[px,g,c] = sum_{ry,rx in 5x5} W[px,g,ry,rx] * Vpad[px+(ry,rx), g,c]
  with w(-1)=relu(-off), w(0)=1-|off|, w(+1)=relu(off) per axis, weighted
  by the softmax mask and accumulated over the 9 kernel points.
- W is built elementwise in a [36=(g,p), px] layout (DVE tensor_scalar
  4x + ACT), scattered to [100=(g,cell), px] via one-hot PE matmuls,
  broadcast over the 16 group-channels by DMA, and applied as 25 fp16
  DVE mult+add passes at 2x rate.
- Apply layout: px in two halves; partitions = (blk2, g4, c16) with
  16-row blocks. Accumulation splits over two DVE chains plus two
  DMA-CCE accumulate chains to hide dependent-op latency; softmax and
  its 9-way broadcast use one-hot matmuls and a broadcast DMA.
"""

import sys

sys.path.insert(0, "/opt/trn_rl_repo")

import numpy as np
import ml_dtypes
from contextlib import ExitStack

import concourse.bass as bass
import concourse.mybir as mybir
from concourse import bacc
from concourse.tile import TileContext
from concourse.bass_utils import run_bass_kernel_spmd

# problem constants (hardcoded per contract)
B, H, W = 4, 128, 128
CIN = 64
G, GC = 4, 16
CH = G * GC          # 64
P = 9                # kernel points
NP = G * P           # 36
NCELL = 25           # 5x5 window
NGCELL = G * NCELL   # 100
N_CORES = 8
ROWS = 64            # output rows per core
PX = ROWS * W        # 8192 output pixels per core
VR = ROWS + 4        # value rows incl +-2 halo = 68
VC = W + 4           # value cols incl halo = 132
CR = ROWS + 2        # conv input rows = 66
CC = W + 4           # conv input cols (padded for pairing) = 132
NBLK, BR = 4, 16     # apply row-blocks

FP = mybir.dt.float32
BF = mybir.dt.float16
NBF = np.float16

_PROGRAM_CACHE = {}


def _build_program(reps=1, debug=False, mode='full'):
    nc = bacc.Bacc("TRN2")

    # ---- DRAM I/O ----
    x1h = nc.dram_tensor("x1h", [CIN, VR * W], BF, kind="ExternalInput")
    x2h = nc.dram_tensor("x2h", [CIN, CR * CC], BF, kind="ExternalInput")
    wv = nc.dram_tensor("wv", [CIN, CH], BF, kind="ExternalInput")
    bv = nc.dram_tensor("bv", [CH, 1], FP, kind="ExternalInput")
    wcp = nc.dram_tensor("wcp", [128, 5 * CH], BF, kind="ExternalInput")
    bconv = nc.dram_tensor("bconv", [CH, 1], FP, kind="ExternalInput")
    whead = nc.dram_tensor("whead", [CIN, 108], BF, kind="ExternalInput")
    bhead = nc.dram_tensor("bhead", [NP, 3], FP, kind="ExternalInput")
    wo2 = nc.dram_tensor("wo2", [128, CH], BF, kind="ExternalInput")
    bo = nc.dram_tensor("bo", [CH, 1], FP, kind="ExternalInput")
    onesg = nc.dram_tensor("onesg", [NP, G], BF, kind="ExternalInput")
    s9 = nc.dram_tensor("s9", [NP, 9 * NGCELL], BF, kind="ExternalInput")
    y = nc.dram_tensor("y", [CH, PX], FP, kind="ExternalOutput")
    if debug:
        dbg_eF = nc.dram_tensor("dbg_eF", [NP, PX], BF, kind="ExternalOutput")
        dbg_off = nc.dram_tensor("dbg_off", [NP, 2 * PX], BF, kind="ExternalOutput")
        dbg_wf = nc.dram_tensor("dbg_wf", [NGCELL, PX], BF, kind="ExternalOutput")
        dbg_acc = nc.dram_tensor("dbg_acc", [128, 2 * BR * W], BF, kind="ExternalOutput")
        dbg_ve = nc.dram_tensor("dbg_ve", [128, 2 * 20 * VC], BF, kind="ExternalOutput")

    AF = mybir.ActivationFunctionType

    with TileContext(nc) as tc:
        with ExitStack() as ctx:
            consts = ctx.enter_context(tc.tile_pool(name="consts", bufs=1))
            persist = ctx.enter_context(tc.tile_pool(name="persist", bufs=1))
            stream = ctx.enter_context(tc.tile_pool(name="stream", bufs=3))
            wbuf = ctx.enter_context(tc.tile_pool(name="wbuf", bufs=2))
            tbuf = ctx.enter_context(tc.tile_pool(name="tbuf", bufs=2))
            psum = ctx.enter_context(tc.tile_pool(name="psum", bufs=1, space="PSUM"))

            # ---- constants to SBUF ----
            wv_t = consts.tile([CIN, CH], BF)
            nc.sync.dma_start(out=wv_t, in_=wv[:, :])
            bv_t = consts.tile([CH, 1], FP)
            nc.sync.dma_start(out=bv_t, in_=bv[:, :])
            wcp_t = consts.tile([128, 5, CH], BF)
            nc.sync.dma_start(out=wcp_t, in_=wcp[:, :].rearrange("k (t m) -> k t m", t=5))
            bconv_t = consts.tile([CH, 1], FP)
            nc.sync.dma_start(out=bconv_t, in_=bconv[:, :])
            whead_t = consts.tile([CIN, 108], BF)
            nc.sync.dma_start(out=whead_t, in_=whead[:, :])
            bhead_t = consts.tile([NP, 3], FP)
            nc.sync.dma_start(out=bhead_t, in_=bhead[:, :])
            wo2_t = consts.tile([128, CH], BF)
            nc.sync.dma_start(out=wo2_t, in_=wo2[:, :])
            bo_t = consts.tile([CH, 1], FP)
            nc.sync.dma_start(out=bo_t, in_=bo[:, :])
            onesg_t = consts.tile([NP, G], BF)
            nc.sync.dma_start(out=onesg_t, in_=onesg[:, :])
            s9_t = consts.tile([NP, 9, NGCELL], BF)
            nc.sync.dma_start(out=s9_t, in_=s9[:, :].rearrange("k (s m) -> k s m", s=9))

            # ---- persistent tiles ----
            # vext per px-half hf: partitions (bb2, g4, c16) where global
            # blk b = 2*hf + bb holds padded-value rows 16b..16b+19.
            vextE = [persist.tile([128, 20, VC], BF, name=f"vextE{hf}")
                     for hf in range(2)]
            vextO = [persist.tile([128, 20, VC], BF, name=f"vextO{hf}")
                     for hf in range(2)]
            for hf in range(2):
                nc.gpsimd.memset(vextE[hf], 0.0)
                nc.gpsimd.memset(vextO[hf], 0.0)
            wfull = persist.tile([NGCELL, PX], BF)
            accs = [persist.tile([128, BR, W], BF, name=f"accs{dd}")
                    for dd in range(2)]
            accd = [persist.tile([128, BR, W], BF, name=f"accd{dd}")
                    for dd in range(2)]
            offF = persist.tile([NP, 2, PX], BF)
            eF = persist.tile([NP, PX], BF)

            for rep in range(reps):
                # ================= value projection =================
                def value_chunk(c):
                    x1c = stream.tile([CIN, 512], BF, tag="x1c", name="x1c", bufs=2)
                    nc.sync.dma_start(out=x1c, in_=x1h[:, c * 512:(c + 1) * 512])
                    psv = psum.tile([CH, 4, W], FP, tag="mmv", bufs=2, name="psv")
                    nc.tensor.matmul(psv, wv_t, x1c, start=True, stop=True)
                    r = 4 * c  # first padded-value row of this chunk
                    for b in range(NBLK):
                        rs, re = max(r, 16 * b), min(r + 4, 16 * b + 20)
                        if rs >= re:
                            continue
                        hf, bb = b // 2, b % 2
                        nc.scalar.activation(
                            vextE[hf][64 * bb:64 * (bb + 1),
                                      rs - 16 * b:re - 16 * b, 2:130],
                            psv[:, rs - r:re - r, :],
                            AF.Identity, bias=bv_t[:, 0:1])

                def odd_copy(hf):
                    nc.gpsimd.tensor_copy(vextO[hf][:, :, 0:131],
                                          vextE[hf][:, :, 1:132])

                # ===== conv + heads (chunks 0-7) with W-build interleaved =====
                def conv_chunk(c):
                    # x2d: lower 64 partitions = x2 rows, upper = shifted +2 cols
                    # x2r: upper = shifted +2 rows (pairs the kx=1 taps)
                    x2d = stream.tile([128, 10, CC], BF, tag="x2d", name="x2d", bufs=2)
                    x2v = x2h[:, :].rearrange("k (r q) -> k r q", q=CC)
                    nc.sync.dma_start(out=x2d[0:64, :, :],
                                      in_=x2v[:, 8 * c:8 * c + 10, :])
                    nc.sync.dma_start(out=x2d[64:128, :, 0:CC - 2],
                                      in_=x2v[:, 8 * c:8 * c + 10, 2:CC])
                    x2r = stream.tile([128, 8, CC], BF, tag="x2r", name="x2r", bufs=2)
                    nc.sync.dma_start(out=x2r[0:64, :, :],
                                      in_=x2v[:, 8 * c:8 * c + 8, :])
                    nc.sync.dma_start(out=x2r[64:128, :, :],
                                      in_=x2v[:, 8 * c + 2:8 * c + 10, :])
                    for s in range(2):
                        cs = 1024 * c + 512 * s
                        psc = psum.tile([CH, 4, W], FP, tag="mm1", bufs=5, name="psc")
                        for kyi in range(3):  # pairs (kyi,0)+(kyi,2)
                            rhs = x2d[:, 4 * s + kyi:4 * s + kyi + 4, 0:W]
                            nc.tensor.matmul(psc, wcp_t[:, kyi, :], rhs,
                                             start=(kyi == 0), stop=False)
                        rhs = x2r[:, 4 * s:4 * s + 4, 1:1 + W]  # (0,1)+(2,1)
                        nc.tensor.matmul(psc, wcp_t[:, 3, :], rhs,
                                         start=False, stop=False)
                        rhs = x2d[0:64, 4 * s + 1:4 * s + 5, 1:1 + W]  # (1,1)
                        nc.tensor.matmul(psc, wcp_t[0:64, 4, :], rhs,
                                         start=False, stop=True)
                        featc = stream.tile([CH, 512], BF, tag="featc", name="featc", bufs=2)
                        nc.scalar.activation(featc, psc.rearrange("p a b -> p (a b)"),
                                             AF.Gelu_apprx_tanh, bias=bconv_t[:, 0:1])
                        for hh in range(3):
                            dst = (offF[:, hh, cs:cs + 512] if hh < 2
                                   else eF[:, cs:cs + 512])
                            psh = psum.tile([NP, 512], FP, tag="mm1", bufs=5, name="psh")
                            nc.tensor.matmul(psh, whead_t[:, 36 * hh:36 * (hh + 1)],
                                             featc, start=True, stop=True)
                            nc.scalar.activation(dst, psh, AF.Identity,
                                                 bias=bhead_t[:, hh:hh + 1])

                def z_normalize(q):
                    # softmax denominator; normalizes eF in place (mask weights)
                    for jj in range(4):
                        js = 2048 * q + 512 * jj
                        psz = psum.tile([G, 512], FP, tag="mm1", bufs=5, name="psz")
                        nc.tensor.matmul(psz, onesg_t, eF[:, js:js + 512],
                                         start=True, stop=True)
                        zi = wbuf.tile([G, 512], BF, tag="zi", name="zi")
                        with nc.allow_low_precision(reason="bf16 softmax denom"):
                            nc.vector.reciprocal(zi, psz)
                        zib = wbuf.tile([NP, 512], BF, tag="zib", name="zib")
                        nc.sync.dma_start(
                            out=zib,
                            in_=zi.unsqueeze(1).broadcast_to([G, P, 512]))
                        nc.vector.tensor_mul(eF[:, js:js + 512],
                                             eF[:, js:js + 512], zib)

                def wbuild_chunk(c):
                    cs = 1024 * c
                    mn = eF[:, cs:cs + 1024]

                    # hat weights via DVE tensor_scalar (4x at bf16)
                    OT = mybir.AluOpType
                    osl = offF[:, :, cs:cs + 1024]
                    rp2 = wbuf.tile([NP, 2, 1024], BF, tag="rp2", name="rp2")
                    nc.vector.tensor_scalar_max(rp2, osl, 0.0)
                    rm2 = wbuf.tile([NP, 2, 1024], BF, tag="rm2", name="rm2")
                    nc.vector.tensor_scalar(rm2, osl, -1.0, 0.0, OT.mult, OT.max)
                    ab2 = wbuf.tile([NP, 2, 1024], BF, tag="ab2", name="ab2")
                    nc.vector.tensor_add(ab2, rp2, rm2)
                    w02 = wbuf.tile([NP, 2, 1024], BF, tag="w02", name="w02")
                    nc.vector.tensor_scalar(w02, ab2, -1.0, 1.0, OT.mult, OT.add)
                    xw = [rm2[:, 0, :], w02[:, 0, :], rp2[:, 0, :]]
                    yw = [rm2[:, 1, :], w02[:, 1, :], rp2[:, 1, :]]

                    my = []
                    for sy in range(3):
                        myt = wbuf.tile([NP, 1024], BF, tag=f"my{sy}", name=f"my{sy}")
                        nc.vector.tensor_mul(myt, mn, yw[sy])
                        my.append(myt)

                    # outer products + scatter to (g, cell), 512-granular psum
                    psws = [psum.tile([NGCELL, 512], FP, tag="mm1", bufs=5,
                                      name=f"psw{j}") for j in range(2)]
                    for sy in range(3):
                        for sx in range(3):
                            wtmp = tbuf.tile([NP, 1024], BF, tag="wtmp", name="wtmp")
                            if sy == 0:
                                nc.gpsimd.tensor_mul(wtmp, my[sy], xw[sx])
                            else:
                                nc.vector.tensor_mul(wtmp, my[sy], xw[sx])
                            si = sy * 3 + sx
                            for j in range(2):
                                nc.tensor.matmul(psws[j], s9_t[:, si, :],
                                                 wtmp[:, 512 * j:512 * (j + 1)],
                                                 start=(si == 0), stop=(si == 8))
                    for j in range(2):
                        nc.scalar.copy(wfull[:, cs + 512 * j:cs + 512 * (j + 1)],
                                       psws[j])

                def apply_cells(hf, cells):
                    # DVE cells: 0-6 split over two accumulator chains;
                    # DMA-CCE cells: 7-24 split over two more chains.
                    if mode == 'dveadds':
                        DMA_CELLS = {}
                        DVE_INIT = (0, 1)
                    else:
                        DMA_CELLS = {c: c % 2 for c in range(7, 23)}
                        DVE_INIT = (0, 1)
                    for cell in cells:
                        ry, rx = cell // 5, cell % 5
                        wexp = tbuf.tile([128, BR, W], BF, tag="wexp", bufs=4,
                                         name="wexp")
                        for bb in range(2):
                            b = 2 * hf + bb
                            src_ = wfull[cell:NGCELL:NCELL, 2048 * b:2048 * (b + 1)]
                            src_ = src_.rearrange("g (r x) -> g r x", r=BR)
                            src_ = src_.unsqueeze(1).broadcast_to([G, GC, BR, W])
                            eng = nc.sync if (cell + bb) % 2 == 0 else nc.scalar
                            eng.dma_start(out=wexp[64 * bb:64 * (bb + 1), :, :],
                                          in_=src_)
                        if rx % 2 == 0:
                            vsl = vextE[hf][:, ry:ry + BR, rx:rx + W]
                        else:
                            vsl = vextO[hf][:, ry:ry + BR, rx - 1:rx - 1 + W]
                        if cell in DVE_INIT:
                            nc.vector.tensor_mul(accs[cell % 2], wexp, vsl)
                        elif cell in (7, 8) and cell in DMA_CELLS:
                            nc.vector.tensor_mul(accd[DMA_CELLS[cell]],
                                                 wexp, vsl)
                        else:
                            tmp = tbuf.tile([128, BR, W], BF, tag="tmp", bufs=4,
                                            name="tmp")
                            nc.vector.tensor_mul(tmp, wexp, vsl)
                            if cell in DMA_CELLS:
                                nc.gpsimd.dma_start(
                                    out=accd[DMA_CELLS[cell]], in_=tmp,
                                    accum_op=mybir.AluOpType.add)
                            else:
                                dst = accs[cell % 2]
                                with nc.allow_low_precision(
                                        reason="bf16 stencil accumulate, checked"):
                                    nc.vector.tensor_add(dst, dst, tmp)

                def apply_merge(hf):
                    srcs = [accs[1]]
                    if mode != 'dveadds':
                        srcs += [accd[0], accd[1]]
                    for t in srcs:
                        with nc.allow_low_precision(
                                reason="bf16 stencil accumulate, checked"):
                            nc.vector.tensor_add(accs[0], accs[0], t)

                def outproj(hf):
                    for bb in range(2):
                        for nq in range(4):
                            pso = psum.tile([CH, 4, W], FP, tag="mm1", bufs=5,
                                            name="pso")
                            nc.tensor.matmul(
                                pso, wo2_t[64 * bb:64 * (bb + 1), :],
                                accs[0][64 * bb:64 * (bb + 1),
                                        4 * nq:4 * (nq + 1), :],
                                start=True, stop=True)
                            outc = stream.tile([CH, 512], FP, tag="outc",
                                               name="outc")
                            nc.scalar.activation(
                                outc, pso.rearrange("p a b -> p (a b)"),
                                AF.Identity, bias=bo_t[:, 0:1])
                            base = (16 * (2 * hf + bb) + 4 * nq) * W
                            nc.sync.dma_start(out=y[:, base:base + 512], in_=outc)

                def exp_z(q):
                    # exp + softmax-normalize for a 2-chunk (2048 px) group
                    nc.scalar.activation(eF[:, 2048 * q:2048 * (q + 1)],
                                         eF[:, 2048 * q:2048 * (q + 1)], AF.Exp)
                    z_normalize(q)

                vc = 0
                for c in range(4):
                    conv_chunk(c)
                    for _ in range(3):
                        if vc < 12:
                            value_chunk(vc); vc += 1
                exp_z(0)
                exp_z(1)
                for c in range(4):
                    conv_chunk(4 + c)
                    if vc < 17:
                        value_chunk(vc); vc += 1
                    if c == 1:
                        odd_copy(0)
                    wbuild_chunk(c)
                while vc < 17:
                    value_chunk(vc); vc += 1
                odd_copy(1)
                exp_z(2)
                exp_z(3)
                cell_slices = [range(0, 6), range(6, 12), range(12, 18),
                               range(18, 25)]
                if mode == 'noapply':
                    for i in range(4):
                        wbuild_chunk(4 + i)
                    for hf in range(2):
                        for bb in range(2):
                            nc.vector.tensor_mul(
                                accs[0][:, 4 * bb:4 * bb + 4, :],
                                vextE[hf][:, 0:4, 2:130],
                                vextE[hf][:, 1:5, 2:130])
                    outproj(0)
                    outproj(1)
                else:
                    for i in range(4):
                        wbuild_chunk(4 + i)
                        apply_cells(0, cell_slices[i])
                    apply_merge(0)
                    outproj(0)
                    apply_cells(1, range(NCELL))
                    apply_merge(1)
                    outproj(1)

                if debug:
                    nc.sync.dma_start(out=dbg_eF[:, :], in_=eF)
                    nc.sync.dma_start(out=dbg_off[:, :],
                                      in_=offF.rearrange("p a b -> p (a b)"))
                    nc.sync.dma_start(out=dbg_wf[:, :], in_=wfull)
                    for hf in range(2):
                        nc.sync.dma_start(
                            out=dbg_acc[:, hf * BR * W:(hf + 1) * BR * W],
                            in_=accs[hf].rearrange("p a b -> p (a b)"))
                        nc.sync.dma_start(
                            out=dbg_ve[:, hf * 20 * VC:(hf + 1) * 20 * VC],
                            in_=vextE[hf].rearrange("p a b -> p (a b)"))

    nc.finalize()
    return nc


def _host_constants(w_value, b_value, w_conv, b_conv, w_offset, b_offset,
                    w_mask, b_mask, w_out, b_out):
    """Shared (per-core identical) small inputs, incl. one-hot helper mats."""
    w_value = np.asarray(w_value, np.float32)
    b_value = np.asarray(b_value, np.float32)
    w_offset = np.asarray(w_offset, np.float32)
    b_offset = np.asarray(b_offset, np.float32)
    w_mask = np.asarray(w_mask, np.float32)
    b_mask = np.asarray(b_mask, np.float32)
    w_out = np.asarray(w_out, np.float32)

    # offset head permutation: col (g*18 + p*2 + xy) -> blocks offx|offy|mask
    idx_x = np.array([g * 18 + p * 2 + 0 for g in range(G) for p in range(P)])
    idx_y = np.array([g * 18 + p * 2 + 1 for g in range(G) for p in range(P)])
    whead = np.concatenate(
        [w_offset[:, idx_x], w_offset[:, idx_y], w_mask], axis=1)
    bhead = np.stack([b_offset[idx_x], b_offset[idx_y], b_mask], axis=1)

    wo2 = np.concatenate([w_out, w_out], axis=0)  # [128, 64]

    # conv tap pairing: wcp[:, t] for t=0..2 stacks taps (t,0) over (t,2);
    # t=3 stacks (0,1) over (2,1); t=4 holds (1,1) in the lower half.
    wc = np.asarray(w_conv, np.float32)  # [3, 3, 64, 64]
    wcp = np.zeros((128, 5, CH), np.float32)
    for t in range(3):
        wcp[0:64, t, :] = wc[t, 0]
        wcp[64:128, t, :] = wc[t, 2]
    wcp[0:64, 3, :] = wc[0, 1]
    wcp[64:128, 3, :] = wc[2, 1]
    wcp[0:64, 4, :] = wc[1, 1]
    wcp = wcp.reshape(128, 5 * CH)

    onesg = np.zeros((NP, G), np.float32)
    for g in range(G):
        for p in range(P):
            onesg[g * 9 + p, g] = 1.0

    s9 = np.zeros((NP, 9, NGCELL), np.float32)
    for sy in range(3):
        for sx in range(3):
            si = sy * 3 + sx
            for g in range(G):
                for kyi in range(3):
                    for kxi in range(3):
                        row = g * 9 + kyi * 3 + kxi
                        cell = (kyi + sy) * 5 + (kxi + sx)
                        s9[row, si, g * NCELL + cell] = 1.0
    s9 = s9.reshape(NP, 9 * NGCELL)

    return {
        "wv": w_value.astype(NBF),
        "bv": b_value[:, None].astype(np.float32),
        "wcp": wcp.astype(NBF),
        "bconv": np.asarray(b_conv, np.float32)[:, None],
        "whead": whead.astype(NBF),
        "bhead": bhead.astype(np.float32),
        "wo2": wo2.astype(NBF),
        "bo": np.asarray(b_out, np.float32)[:, None],
        "onesg": onesg.astype(NBF),
        "s9": s9.astype(NBF),
    }


def _per_core_inputs(x1, x2, shared):
    """Slice + zero-pad the two activation streams per core."""
    x1 = np.asarray(x1, np.float32)
    x2 = np.asarray(x2, np.float32)
    in_maps = []
    for core in range(N_CORES):
        b, half = core // 2, core % 2
        r0 = ROWS * half
        x1p = np.zeros((CIN, VR, W), np.float32)
        lo, hi = r0 - 2, r0 + 66
        slo, shi = max(lo, 0), min(hi, H)
        x1p[:, slo - lo:shi - lo, :] = x1[b, :, slo:shi, :]
        x2p = np.zeros((CIN, CR, CC), np.float32)
        lo2, hi2 = r0 - 1, r0 + 65
        slo2, shi2 = max(lo2, 0), min(hi2, H)
        x2p[:, slo2 - lo2:shi2 - lo2, 1:1 + W] = x2[b, :, slo2:shi2, :]
        m = {"x1h": x1p.reshape(CIN, VR * W).astype(NBF),
             "x2h": x2p.reshape(CIN, CR * CC).astype(NBF)}
        m.update(shared)
        in_maps.append(m)
    return in_maps


def _get_program(reps=1, mode='full'):
    key = (reps, mode)
    if key not in _PROGRAM_CACHE:
        _PROGRAM_CACHE[key] = _build_program(reps, mode=mode)
    return _PROGRAM_CACHE[key]


def kernel(x1, x2, w_value, b_value, w_conv, b_conv, w_offset, b_offset,
           w_mask, b_mask, w_out, b_out):
    shared = _host_constants(w_value, b_value, w_conv, b_conv, w_offset,
                             b_offset, w_mask, b_mask, w_out, b_out)
    in_maps = _per_core_inputs(x1, x2, shared)
    nc = _get_program(reps=1)
    res = run_bass_kernel_spmd(nc, in_maps, list(range(N_CORES)))
    out = np.empty((B, CH, H, W), np.float32)
    for core in range(N_CORES):
        b, half = core // 2, core % 2
        out[b, :, ROWS * half:ROWS * (half + 1), :] = (
            res.results[core]["y"].reshape(CH, ROWS, W))
    return out


def run_for_timing(inputs, reps):
    """Used by test.py: run the reps-unrolled program once, return results."""
    shared = _host_constants(
        inputs["w_value"], inputs["b_value"], inputs["w_conv"], inputs["b_conv"],
        inputs["w_offset"], inputs["b_offset"], inputs["w_mask"], inputs["b_mask"],
        inputs["w_out"], inputs["b_out"])
    in_maps = _per_core_inputs(inputs["x1"], inputs["x2"], shared)
    nc = _get_program(reps=reps)
    return run_bass_kernel_spmd(nc, in_maps, list(range(N_CORES)))

